# revision 2
# baseline (speedup 1.0000x reference)
"""CapsuleFC kernel for 8 trn2 NeuronCores.

Shards the output-capsule dim (N_OUT=128 -> 16 per core). W is pre-sliced,
pre-transposed and cast to fp16 on the host; each core streams its 67MB W
shard from HBM once (the wall-clock floor), computing

  votes[b,n,m,d] = sum_a x[b,n,a] W[n,a,m,d]

as block-diagonal accumulated matmuls: groups of 8 input capsules n share one
PSUM tile with partition layout (n8, b) = 128. The routing softmax over m
needs one 16KB AllReduce of per-shard exp-sums; everything else is local.
"""

import sys

if "/opt/trn_rl_repo" not in sys.path:
    sys.path.insert(0, "/opt/trn_rl_repo")

import numpy as np

B, NIN, A, MTOT, D = 16, 256, 128, 128, 64
NCORES = 8
MLOC = MTOT // NCORES      # 16 output capsules per core
K8 = 8                     # input capsules per matmul group
G = NIN // K8              # 32 groups
MD = MLOC * D              # 1024 free columns per group
F16 = np.float16

_CACHE = {}


def _build_nc():
    import concourse.bacc as bacc
    import concourse.mybir as mybir
    from concourse import tile

    f16 = mybir.dt.float16
    f32 = mybir.dt.float32

    nc = bacc.Bacc(None, target_bir_lowering=False)

    w_d = nc.dram_tensor("w", [G, 128, K8 * MD], f16, kind="ExternalInput")
    xp_d = nc.dram_tensor("xp", [G, 128, K8 * 128], f16, kind="ExternalInput")
    ones_d = nc.dram_tensor("ones16", [128, 16], f16, kind="ExternalInput")
    rep_d = nc.dram_tensor("rep16", [16, 128], f16, kind="ExternalInput")
    actr_d = nc.dram_tensor("actr", [128, G], f32, kind="ExternalInput")
    cfac_d = nc.dram_tensor("cfac", [128, 1], f32, kind="ExternalInput")
    qk_d = nc.dram_tensor("qk_sh", [K8, B, G, MLOC], f32, kind="ExternalOutput")
    out_d = nc.dram_tensor("out_sh", [B, MLOC, D], f32, kind="ExternalOutput")

    AX = mybir.AxisListType.X
    ADD = mybir.AluOpType.add
    MUL = mybir.AluOpType.mult
    EXP = mybir.ActivationFunctionType.Exp
    CPY = mybir.ActivationFunctionType.Copy

    with tile.TileContext(nc) as tc:
        with (
            tc.tile_pool(name="wp", bufs=3) as wp,
            tc.tile_pool(name="xpp", bufs=3) as xpp,
            tc.tile_pool(name="vsb", bufs=1) as vsb,
            tc.tile_pool(name="sb", bufs=1) as sb,
            tc.tile_pool(name="tmp", bufs=2) as tmp,
            tc.tile_pool(name="vp", bufs=4, space="PSUM") as vp,
            tc.tile_pool(name="accp", bufs=1, space="PSUM") as accp,
            tc.tile_pool(name="dram", bufs=2, space="DRAM") as dram,
        ):
            ones16 = sb.tile([128, 16], f16, tag="ones16")
            rep16 = sb.tile([16, 128], f16, tag="rep16")
            actr = sb.tile([128, G], f32, tag="actr")
            cfac = sb.tile([128, 1], f32, tag="cfac")
            nc.sync.dma_start(ones16[:], ones_d[:])
            nc.sync.dma_start(rep16[:], rep_d[:])
            nc.sync.dma_start(actr[:], actr_d[:])
            nc.sync.dma_start(cfac[:], cfac_d[:])

            votes = vsb.tile([128, G * MD], f16, tag="votes")
            ncvp = accp.tile([16, MD], f32, tag="ncvp")
            outp = accp.tile([16, MD], f32, tag="outp")

            # ---- phase 1: stream W, build votes, accumulate ncv ----
            for g in range(G):
                w_t = wp.tile([128, K8 * MD], f16, tag="w")
                x_t = xpp.tile([128, K8 * 128], f16, tag="x")
                nc.sync.dma_start(w_t[:], w_d[g])
                nc.sync.dma_start(x_t[:], xp_d[g])

                vps = [vp.tile([128, 512], f32, tag="vp", name=f"vp{g}_{i}") for i in range(2)]
                for k in range(K8):
                    lhsT = x_t[:, k * 128:(k + 1) * 128]
                    for c in range(2):
                        nc.tensor.matmul(
                            vps[c][:],
                            lhsT,
                            w_t[:, k * MD + c * 512: k * MD + (c + 1) * 512],
                            start=(k == 0),
                            stop=(k == K8 - 1),
                        )
                for c in range(2):
                    vg = votes[:, g * MD + c * 512: g * MD + (c + 1) * 512]
                    nc.scalar.copy(vg, vps[c][:])
                    nc.tensor.matmul(
                        ncvp[:, c * 512:(c + 1) * 512],
                        ones16[:],
                        vg,
                        start=(g == 0),
                        stop=(g == G - 1),
                    )

            # ---- tail: scores = sum_d votes * ncv/1024 ----
            ncv16 = sb.tile([16, MD], f16, tag="ncv16")
            nc.scalar.activation(ncv16[:], ncvp[:], CPY, scale=1.0 / (MTOT * 8.0))
            ncv_bc = sb.tile([128, MD], f16, tag="ncv_bc")
            for c in range(2):
                nb = vp.tile([128, 512], f32, tag="vp")
                nc.tensor.matmul(
                    nb[:], rep16[:], ncv16[:, c * 512:(c + 1) * 512],
                    start=True, stop=True,
                )
                nc.vector.tensor_copy(ncv_bc[:, c * 512:(c + 1) * 512], nb[:])

            scores = sb.tile([128, G * MLOC], f32, tag="scores")
            for g in range(G):
                t = tmp.tile([128, MD], f16, tag="t")
                nc.vector.tensor_mul(t[:], votes[:, g * MD:(g + 1) * MD], ncv_bc[:])
                nc.vector.tensor_reduce(
                    scores[:, g * MLOC:(g + 1) * MLOC],
                    t.rearrange("p (m d) -> p m d", d=D)[:],
                    axis=AX, op=ADD,
                )

            # ---- softmax over m (global over all shards via AllReduce) ----
            ev = sb.tile([128, G * MLOC], f32, tag="ev")
            nc.scalar.activation(ev[:], scores[:], EXP)
            zp = sb.tile([128, G], f32, tag="zp")
            nc.vector.tensor_reduce(
                zp[:], ev.rearrange("p (g m) -> p g m", m=MLOC)[:], axis=AX, op=ADD
            )
            zin = dram.tile([128, G], f32, tag="zin")
            zout = dram.tile([128, G], f32, tag="zout")
            nc.sync.dma_start(zin[:], zp[:])
            nc.gpsimd.collective_compute(
                "AllReduce",
                ADD,
                replica_groups=[list(range(NCORES))],
                ins=[zin.opt()],
                outs=[zout.opt()],
            )
            zt = sb.tile([128, G], f32, tag="zt")
            nc.sync.dma_start(zt[:], zout[:])
            zr = sb.tile([128, G], f32, tag="zr")
            nc.vector.reciprocal(zr[:], zt[:])

            # qk = ev * zr (bcast over m) * cfac  (cfac folds next_act/(.+eps))
            qk = sb.tile([128, G * MLOC], f32, tag="qk")
            zr_b = zr.unsqueeze(2).broadcast_to([128, G, MLOC])
            nc.vector.scalar_tensor_tensor(
                qk.rearrange("p (g m) -> p g m", m=MLOC)[:],
                ev.rearrange("p (g m) -> p g m", m=MLOC)[:],
                cfac[:],
                zr_b,
                op0=MUL, op1=MUL,
            )
            nc.sync.dma_start(qk_d[:], qk.rearrange("p (g m) -> p g m", m=MLOC)[:])

            # ---- weighted reduce: out[b,m,d] = sum_n qk*act*votes ----
            wq = sb.tile([128, G * MLOC], f16, tag="wq")
            nc.vector.tensor_mul(
                wq.rearrange("p (g m) -> p g m", m=MLOC)[:],
                qk.rearrange("p (g m) -> p g m", m=MLOC)[:],
                actr.unsqueeze(2).broadcast_to([128, G, MLOC]),
            )
            for g in range(G):
                wv = tmp.tile([128, MD], f16, tag="t")
                nc.vector.tensor_mul(
                    wv.rearrange("p (m d) -> p m d", d=D)[:],
                    votes[:, g * MD:(g + 1) * MD].rearrange(
                        "p (m d) -> p m d", d=D
                    ),
                    wq[:, g * MLOC:(g + 1) * MLOC].unsqueeze(2).broadcast_to(
                        [128, MLOC, D]
                    ),
                )
                for c in range(2):
                    nc.tensor.matmul(
                        outp[:, c * 512:(c + 1) * 512],
                        ones16[:],
                        wv[:, c * 512:(c + 1) * 512],
                        start=(g == 0),
                        stop=(g == G - 1),
                    )

            osb = sb.tile([16, MD], f32, tag="osb")
            nc.vector.tensor_copy(osb[:], outp[:])
            nc.sync.dma_start(out_d.rearrange("b m d -> b (m d)"), osb[:])

    nc.finalize()
    return nc


def _get_nc():
    if "nc" not in _CACHE:
        _CACHE["nc"] = _build_nc()
    return _CACHE["nc"]


def _host_inputs(x, act, W):
    """Build per-core device input maps."""
    x = np.asarray(x, np.float32)
    act = np.asarray(act, np.float32)
    W = np.asarray(W, np.float32)

    # W[n,a,m,d] -> per-core [g, a, (k, m, d)], n = 8g + k, m = 16*core + m_loc
    W6 = W.reshape(G, K8, A, NCORES, MLOC, D)
    Wt = np.ascontiguousarray(W6.transpose(3, 0, 2, 1, 4, 5), dtype=F16)
    Wt = Wt.reshape(NCORES, G, A, K8 * MD)

    # block-diagonal lhsT slabs: xp[g, a, k*128 + 16k+b] = x[b, 8g+k, a]
    xg = np.ascontiguousarray(x.transpose(1, 2, 0), dtype=F16).reshape(G, K8, A, B)
    xp = np.zeros((G, A, K8, 128), F16)
    for k in range(K8):
        xp[:, :, k, 16 * k:16 * (k + 1)] = xg[:, k]
    xp = xp.reshape(G, A, K8 * 128)

    ones16 = np.tile(np.eye(16, dtype=F16), (K8, 1))
    rep16 = np.tile(np.eye(16, dtype=F16), (1, K8))
    actr = np.ascontiguousarray(
        act.reshape(B, G, K8).transpose(2, 0, 1), dtype=np.float32
    ).reshape(128, G)
    c = act.mean(axis=1)
    cfac = (c.astype(np.float64) / (c.astype(np.float64) + 1e-10)).astype(np.float32)
    cfac_t = np.tile(cfac, K8).reshape(128, 1)

    common = {
        "xp": xp, "ones16": ones16, "rep16": rep16,
        "actr": actr, "cfac": cfac_t,
    }
    in_maps = [{"w": np.ascontiguousarray(Wt[cid]), **common} for cid in range(NCORES)]
    return in_maps, c


def kernel(input, current_act, W, num_iter=1, **_unused):
    from concourse.bass_utils import run_bass_kernel_spmd

    in_maps, c = _host_inputs(input, current_act, W)
    nc = _get_nc()
    res = run_bass_kernel_spmd(nc, in_maps, list(range(NCORES)))

    next_capsule_value = np.concatenate(
        [res.results[cid]["out_sh"] for cid in range(NCORES)], axis=1
    ).astype(np.float32, copy=False)
    qk = np.concatenate(
        [
            res.results[cid]["qk_sh"].transpose(1, 2, 0, 3).reshape(B, NIN, MLOC)
            for cid in range(NCORES)
        ],
        axis=2,
    ).astype(np.float32, copy=False)
    next_act = np.ascontiguousarray(
        np.broadcast_to(c[:, None].astype(np.float32), (B, MTOT))
    )
    return (next_capsule_value, next_act, qk)


# revision 12
# speedup vs baseline: 1.0373x; 1.0373x over previous
"""CapsuleFC kernel for 8 trn2 NeuronCores.

Shards the output-capsule dim (N_OUT=128 -> 16 per core). W is pre-sliced,
pre-transposed and cast to fp16 on the host; each core streams its 67MB W
shard from HBM once (the wall-clock floor), computing

  votes[b,n,m,d] = sum_a x[b,n,a] W[n,a,m,d]

as block-diagonal accumulated matmuls: groups of 8 input capsules n share one
PSUM tile with partition layout (n8, b) = 128. The routing softmax over m
needs one 16KB AllReduce of per-shard exp-sums; everything else is local.
"""

import sys

if "/opt/trn_rl_repo" not in sys.path:
    sys.path.insert(0, "/opt/trn_rl_repo")

import numpy as np

B, NIN, A, MTOT, D = 16, 256, 128, 128, 64
NCORES = 8
MLOC = MTOT // NCORES      # 16 output capsules per core
K8 = 8                     # input capsules per matmul group
G = NIN // K8              # 32 groups
MD = MLOC * D              # 1024 free columns per group
F16 = np.float16

_CACHE = {}


def _build_nc():
    import concourse.bacc as bacc
    import concourse.mybir as mybir
    from concourse import tile

    f16 = mybir.dt.float16
    f32 = mybir.dt.float32

    nc = bacc.Bacc(None, target_bir_lowering=False)

    w_d = nc.dram_tensor("w", [G, 128, K8 * MD], f16, kind="ExternalInput")
    xall_d = nc.dram_tensor("xall", [128, G * 128], f16, kind="ExternalInput")
    ones_d = nc.dram_tensor("ones16", [128, 16], f16, kind="ExternalInput")
    rep_d = nc.dram_tensor("rep16", [16, 128], f16, kind="ExternalInput")
    actr_d = nc.dram_tensor("actr", [128, G], f32, kind="ExternalInput")
    cfac_d = nc.dram_tensor("cfac", [128, 1], f32, kind="ExternalInput")
    qk_d = nc.dram_tensor("qk_sh", [K8, B, G, MLOC], f32, kind="ExternalOutput")
    out_d = nc.dram_tensor("out_sh", [B, MLOC, D], f32, kind="ExternalOutput")

    AX = mybir.AxisListType.X
    ADD = mybir.AluOpType.add
    MUL = mybir.AluOpType.mult
    EXP = mybir.ActivationFunctionType.Exp
    CPY = mybir.ActivationFunctionType.Copy

    with tile.TileContext(nc) as tc:
        with (
            tc.tile_pool(name="wp", bufs=3) as wp,
            tc.tile_pool(name="xpp", bufs=3) as xpp,
            tc.tile_pool(name="vsb", bufs=1) as vsb,
            tc.tile_pool(name="sb", bufs=1) as sb,
            tc.tile_pool(name="tmp", bufs=2) as tmp,
            tc.tile_pool(name="vp", bufs=4, space="PSUM") as vp,
            tc.tile_pool(name="accp", bufs=1, space="PSUM") as accp,
            tc.tile_pool(name="dram", bufs=2, space="DRAM") as dram,
        ):
            ones16 = sb.tile([128, 16], f16, tag="ones16")
            rep16 = sb.tile([16, 128], f16, tag="rep16")
            actr = sb.tile([128, G], f32, tag="actr")
            cfac = sb.tile([128, 1], f32, tag="cfac")
            nc.sync.dma_start(ones16[:], ones_d[:])
            nc.sync.dma_start(rep16[:], rep_d[:])
            nc.sync.dma_start(actr[:], actr_d[:])
            nc.sync.dma_start(cfac[:], cfac_d[:])

            votes = vsb.tile([128, G * MD], f16, tag="votes")
            ncvp = accp.tile([16, MD], f32, tag="ncvp")
            outp = accp.tile([16, MD], f32, tag="outp")

            # dense x, one upload; block-diagonal lhsT slabs built on-device.
            # slab block k = cols [128k, 128k+128); its 16 data cols sit at
            # 144k = 128k + 16k, everything else stays zero after the memset.
            xall = xpp.tile([128, G * 128], f16, tag="xall")
            nc.sync.dma_start(xall[:], xall_d[:])
            slabs = []
            for i in range(3):
                s = xpp.tile([128, K8 * 128], f16, tag=f"slab{i}", name=f"slab{i}")
                nc.gpsimd.memset(s[:], 0.0)
                slabs.append(s)

            # ---- phase 1: stream W, build votes, accumulate ncv ----
            for g in range(G):
                w_t = wp.tile([128, K8 * MD], f16, tag="w")
                nc.sync.dma_start(w_t[:, :K8 * MD // 2], w_d[g, :, :K8 * MD // 2])
                nc.sync.dma_start(w_t[:, K8 * MD // 2:], w_d[g, :, K8 * MD // 2:])
                x_t = slabs[g % 3]
                for k in range(K8):
                    nc.vector.tensor_copy(
                        x_t[:, 144 * k:144 * k + 16],
                        xall[:, 128 * g + 16 * k:128 * g + 16 * k + 16],
                    )

                vps = [vp.tile([128, 512], f32, tag="vp", name=f"vp{g}_{i}") for i in range(2)]
                for k in range(K8):
                    lhsT = x_t[:, k * 128:(k + 1) * 128]
                    for c in range(2):
                        nc.tensor.matmul(
                            vps[c][:],
                            lhsT,
                            w_t[:, k * MD + c * 512: k * MD + (c + 1) * 512],
                            start=(k == 0),
                            stop=(k == K8 - 1),
                        )
                for c in range(2):
                    vg = votes[:, g * MD + c * 512: g * MD + (c + 1) * 512]
                    nc.scalar.copy(vg, vps[c][:])
                    nc.tensor.matmul(
                        ncvp[:, c * 512:(c + 1) * 512],
                        ones16[:],
                        vg,
                        start=(g == 0),
                        stop=(g == G - 1),
                    )

            # ---- tail: scores = sum_d votes * ncv/1024 ----
            ncv16 = sb.tile([16, MD], f16, tag="ncv16")
            nc.scalar.activation(ncv16[:], ncvp[:], CPY, scale=1.0 / (MTOT * 8.0))
            ncv_bc = sb.tile([128, MD], f16, tag="ncv_bc")
            for c in range(2):
                nb = vp.tile([128, 512], f32, tag="vp")
                nc.tensor.matmul(
                    nb[:], rep16[:], ncv16[:, c * 512:(c + 1) * 512],
                    start=True, stop=True,
                )
                nc.vector.tensor_copy(ncv_bc[:, c * 512:(c + 1) * 512], nb[:])

            scores = sb.tile([128, G * MLOC], f32, tag="scores")
            for g in range(G):
                t = tmp.tile([128, MD], f16, tag="t")
                nc.gpsimd.tensor_mul(t[:], votes[:, g * MD:(g + 1) * MD], ncv_bc[:])
                nc.vector.tensor_reduce(
                    scores[:, g * MLOC:(g + 1) * MLOC],
                    t.rearrange("p (m d) -> p m d", d=D)[:],
                    axis=AX, op=ADD,
                )
                # keep the PE HAM clock-gate warm through the DVE-bound
                # stretch: a junk matmul paced by each group's product tile
                jp = vp.tile([128, 512], f32, tag="vp", name=f"warm{g}")
                nc.tensor.matmul(jp[:16, :], ones16[:], t[:, :512],
                                 start=True, stop=True)

            # ---- softmax over m (global over all shards via AllReduce) ----
            ev = sb.tile([128, G * MLOC], f32, tag="ev")
            nc.scalar.activation(ev[:], scores[:], EXP)
            zp = sb.tile([128, G], f32, tag="zp")
            nc.vector.tensor_reduce(
                zp[:], ev.rearrange("p (g m) -> p g m", m=MLOC)[:], axis=AX, op=ADD
            )
            zin = dram.tile([128, G], f32, tag="zin")
            zout = dram.tile([128, G], f32, tag="zout")
            nc.sync.dma_start(zin[:], zp[:])
            nc.gpsimd.collective_compute(
                "AllReduce",
                ADD,
                replica_groups=[list(range(NCORES))],
                ins=[zin.opt()],
                outs=[zout.opt()],
            )

            # ---- work hidden under the AllReduce ----
            # u = ev * act * cfac; votes *= u (in place, bcast over d). The
            # missing 1/z factor rides in the matmul lhsT (zdiag) post-AR.
            u = sb.tile([128, G * MLOC], f16, tag="u")
            nc.vector.scalar_tensor_tensor(
                u.rearrange("p (g m) -> p g m", m=MLOC)[:],
                ev.rearrange("p (g m) -> p g m", m=MLOC)[:],
                cfac[:],
                actr.unsqueeze(2).broadcast_to([128, G, MLOC]),
                op0=MUL, op1=MUL,
            )
            for g in range(G):
                vg3 = votes[:, g * MD:(g + 1) * MD].rearrange(
                    "p (m d) -> p m d", d=D
                )
                nc.vector.tensor_mul(
                    vg3,
                    vg3,
                    u[:, g * MLOC:(g + 1) * MLOC].unsqueeze(2).broadcast_to(
                        [128, MLOC, D]
                    ),
                )
            # HAM warmers paced by an ACT<->PE chain spanning the AR wait
            jk = sb.tile([128, 512], f16, tag="jk")
            nc.scalar.copy(jk[:], ev[:, :512])
            for i in range(14):
                jp = vp.tile([128, 512], f32, tag="vp", name=f"arwarm{i}")
                nc.tensor.matmul(jp[:16, :], ones16[:], jk[:, :512],
                                 start=True, stop=True)
                nc.scalar.copy(jk[:16, :], jp[:16, :])

            zt = sb.tile([128, G], f32, tag="zt")
            nc.sync.dma_start(zt[:], zout[:])
            zr = sb.tile([128, G], f32, tag="zr")
            nc.vector.reciprocal(zr[:], zt[:])

            # qk = ev * zr (bcast over m) * cfac  (cfac folds next_act/(.+eps))
            qk = sb.tile([128, G * MLOC], f32, tag="qk")
            zr_b = zr.unsqueeze(2).broadcast_to([128, G, MLOC])
            nc.vector.scalar_tensor_tensor(
                qk.rearrange("p (g m) -> p g m", m=MLOC)[:],
                ev.rearrange("p (g m) -> p g m", m=MLOC)[:],
                cfac[:],
                zr_b,
                op0=MUL, op1=MUL,
            )
            nc.sync.dma_start(qk_d[:], qk.rearrange("p (g m) -> p g m", m=MLOC)[:])

            # ---- weighted reduce: out[b,m,d] = sum_n zr * votes' where
            # votes' = votes*ev*act*cfac; zdiag[(k,b), b'] = d(b,b') zr[kb,g]
            zdiag = sb.tile([128, G * MLOC], f16, tag="zdiag")
            nc.vector.tensor_mul(
                zdiag.rearrange("p (g m) -> p g m", m=MLOC)[:],
                ones16.unsqueeze(1).broadcast_to([128, G, MLOC]),
                zr_b,
            )
            for g in range(G):
                for c in range(2):
                    nc.tensor.matmul(
                        outp[:, c * 512:(c + 1) * 512],
                        zdiag[:, g * MLOC:(g + 1) * MLOC],
                        votes[:, g * MD + c * 512: g * MD + (c + 1) * 512],
                        start=(g == 0),
                        stop=(g == G - 1),
                    )

            osb = sb.tile([16, MD], f32, tag="osb")
            nc.vector.tensor_copy(osb[:], outp[:])
            nc.sync.dma_start(out_d.rearrange("b m d -> b (m d)"), osb[:])

    nc.finalize()
    return nc


def _get_nc():
    if "nc" not in _CACHE:
        _CACHE["nc"] = _build_nc()
    return _CACHE["nc"]


def _host_inputs(x, act, W):
    """Build per-core device input maps."""
    x = np.asarray(x, np.float32)
    act = np.asarray(act, np.float32)
    W = np.asarray(W, np.float32)

    # W[n,a,m,d] -> per-core [g, a, (k, m, d)], n = 8g + k, m = 16*core + m_loc
    W6 = W.reshape(G, K8, A, NCORES, MLOC, D)
    Wt = np.ascontiguousarray(W6.transpose(3, 0, 2, 1, 4, 5), dtype=F16)
    Wt = Wt.reshape(NCORES, G, A, K8 * MD)

    # dense x with (g, k, b) on the free dim: xall[a, 128g+16k+b] = x[b, 8g+k, a]
    xall = np.ascontiguousarray(x.transpose(2, 1, 0), dtype=F16).reshape(A, G * 128)

    ones16 = np.tile(np.eye(16, dtype=F16), (K8, 1))
    rep16 = np.tile(np.eye(16, dtype=F16), (1, K8))
    actr = np.ascontiguousarray(
        act.reshape(B, G, K8).transpose(2, 0, 1), dtype=np.float32
    ).reshape(128, G)
    c = act.mean(axis=1)
    cfac = (c.astype(np.float64) / (c.astype(np.float64) + 1e-10)).astype(np.float32)
    cfac_t = np.tile(cfac, K8).reshape(128, 1)

    common = {
        "xall": xall, "ones16": ones16, "rep16": rep16,
        "actr": actr, "cfac": cfac_t,
    }
    in_maps = [{"w": np.ascontiguousarray(Wt[cid]), **common} for cid in range(NCORES)]
    return in_maps, c


def kernel(input, current_act, W, num_iter=1, **_unused):
    from concourse.bass_utils import run_bass_kernel_spmd

    in_maps, c = _host_inputs(input, current_act, W)
    nc = _get_nc()
    res = run_bass_kernel_spmd(nc, in_maps, list(range(NCORES)))

    next_capsule_value = np.concatenate(
        [res.results[cid]["out_sh"] for cid in range(NCORES)], axis=1
    ).astype(np.float32, copy=False)
    qk = np.concatenate(
        [
            res.results[cid]["qk_sh"].transpose(1, 2, 0, 3).reshape(B, NIN, MLOC)
            for cid in range(NCORES)
        ],
        axis=2,
    ).astype(np.float32, copy=False)
    next_act = np.ascontiguousarray(
        np.broadcast_to(c[:, None].astype(np.float32), (B, MTOT))
    )
    return (next_capsule_value, next_act, qk)


# revision 13
# speedup vs baseline: 1.0827x; 1.0437x over previous
"""CapsuleFC kernel for 8 trn2 NeuronCores.

Shards the output-capsule dim (N_OUT=128 -> 16 per core). W is pre-sliced,
pre-transposed and cast to fp16 on the host; each core streams its 67MB W
shard from HBM once (the wall-clock floor), computing

  votes[b,n,m,d] = sum_a x[b,n,a] W[n,a,m,d]

as block-diagonal accumulated matmuls: groups of 8 input capsules n share one
PSUM tile with partition layout (n8, b) = 128. The routing softmax over m
needs one 16KB AllReduce of per-shard exp-sums; everything else is local.
"""

import sys

if "/opt/trn_rl_repo" not in sys.path:
    sys.path.insert(0, "/opt/trn_rl_repo")

import numpy as np

B, NIN, A, MTOT, D = 16, 256, 128, 128, 64
NCORES = 8
MLOC = MTOT // NCORES      # 16 output capsules per core
K8 = 8                     # input capsules per matmul group
G = NIN // K8              # 32 groups
MD = MLOC * D              # 1024 free columns per group
F16 = np.float16

_CACHE = {}


def _build_nc():
    import concourse.bacc as bacc
    import concourse.mybir as mybir
    from concourse import tile

    f16 = mybir.dt.float16
    f32 = mybir.dt.float32

    nc = bacc.Bacc(None, target_bir_lowering=False)

    w_d = nc.dram_tensor("w", [G, 128, K8 * MD], f16, kind="ExternalInput")
    xall_d = nc.dram_tensor("xall", [128, G * 128], f16, kind="ExternalInput")
    ones_d = nc.dram_tensor("ones16", [128, 16], f16, kind="ExternalInput")
    rep_d = nc.dram_tensor("rep16", [16, 128], f16, kind="ExternalInput")
    actr_d = nc.dram_tensor("actr", [128, G], f32, kind="ExternalInput")
    cfac_d = nc.dram_tensor("cfac", [128, 1], f32, kind="ExternalInput")
    qk_d = nc.dram_tensor("qk_sh", [K8, B, G, MLOC], f32, kind="ExternalOutput")
    out_d = nc.dram_tensor("out_sh", [B, MLOC, D], f32, kind="ExternalOutput")

    AX = mybir.AxisListType.X
    ADD = mybir.AluOpType.add
    MUL = mybir.AluOpType.mult
    EXP = mybir.ActivationFunctionType.Exp
    CPY = mybir.ActivationFunctionType.Copy

    with tile.TileContext(nc) as tc:
        with (
            tc.tile_pool(name="wp", bufs=3) as wp,
            tc.tile_pool(name="xpp", bufs=3) as xpp,
            tc.tile_pool(name="vsb", bufs=1) as vsb,
            tc.tile_pool(name="sb", bufs=1) as sb,
            tc.tile_pool(name="tmp", bufs=2) as tmp,
            tc.tile_pool(name="vp", bufs=4, space="PSUM") as vp,
            tc.tile_pool(name="accp", bufs=1, space="PSUM") as accp,
            tc.tile_pool(name="dram", bufs=2, space="DRAM") as dram,
        ):
            ones16 = sb.tile([128, 16], f16, tag="ones16")
            rep16 = sb.tile([16, 128], f16, tag="rep16")
            actr = sb.tile([128, G], f32, tag="actr")
            cfac = sb.tile([128, 1], f32, tag="cfac")
            nc.sync.dma_start(ones16[:], ones_d[:])
            nc.sync.dma_start(rep16[:], rep_d[:])
            nc.sync.dma_start(actr[:], actr_d[:])
            nc.sync.dma_start(cfac[:], cfac_d[:])

            votes = vsb.tile([128, G * MD], f16, tag="votes")
            ncvp = accp.tile([16, MD], f32, tag="ncvp")
            outp = accp.tile([16, MD], f32, tag="outp")

            # dense x, one upload; block-diagonal lhsT slabs built on-device.
            # slab block k = cols [128k, 128k+128); its 16 data cols sit at
            # 144k = 128k + 16k, everything else stays zero after the memset.
            xall = xpp.tile([128, G * 128], f16, tag="xall")
            nc.sync.dma_start(xall[:], xall_d[:])
            slabs = []
            for i in range(3):
                s = xpp.tile([128, K8 * 128], f16, tag=f"slab{i}", name=f"slab{i}")
                nc.gpsimd.memset(s[:], 0.0)
                slabs.append(s)

            # ---- phase 1: stream W, build votes, accumulate ncv ----
            for g in range(G):
                w_t = wp.tile([128, K8 * MD], f16, tag="w")
                nc.sync.dma_start(w_t[:, :K8 * MD // 2], w_d[g, :, :K8 * MD // 2])
                nc.sync.dma_start(w_t[:, K8 * MD // 2:], w_d[g, :, K8 * MD // 2:])
                x_t = slabs[g % 3]
                for k in range(K8):
                    nc.vector.tensor_copy(
                        x_t[:, 144 * k:144 * k + 16],
                        xall[:, 128 * g + 16 * k:128 * g + 16 * k + 16],
                    )

                vps = [vp.tile([128, 512], f32, tag="vp", name=f"vp{g}_{i}") for i in range(2)]
                for k in range(K8):
                    lhsT = x_t[:, k * 128:(k + 1) * 128]
                    for c in range(2):
                        nc.tensor.matmul(
                            vps[c][:],
                            lhsT,
                            w_t[:, k * MD + c * 512: k * MD + (c + 1) * 512],
                            start=(k == 0),
                            stop=(k == K8 - 1),
                        )
                for c in range(2):
                    vg = votes[:, g * MD + c * 512: g * MD + (c + 1) * 512]
                    nc.scalar.copy(vg, vps[c][:])
                    nc.tensor.matmul(
                        ncvp[:, c * 512:(c + 1) * 512],
                        ones16[:],
                        vg,
                        start=(g == 0),
                        stop=(g == G - 1),
                    )

            # ---- tail: scores = sum_d votes * ncv/1024 ----
            ncv16 = sb.tile([16, MD], f16, tag="ncv16")
            nc.scalar.activation(ncv16[:], ncvp[:], CPY, scale=1.0 / (MTOT * 8.0))
            ncv_bc = sb.tile([128, MD], f16, tag="ncv_bc")
            for c in range(2):
                nb = vp.tile([128, 512], f32, tag="vp")
                nc.tensor.matmul(
                    nb[:], rep16[:], ncv16[:, c * 512:(c + 1) * 512],
                    start=True, stop=True,
                )
                nc.vector.tensor_copy(ncv_bc[:, c * 512:(c + 1) * 512], nb[:])

            scores = sb.tile([128, G * MLOC], f32, tag="scores")
            for g in range(G):
                t = tmp.tile([128, MD], f16, tag="t")
                # measured: gpsimd TT ~2.1us, DVE TT ~0.95us + 32 reduces;
                # 23/9 split equalizes the two queues at ~48us
                teng = nc.gpsimd if g < 23 else nc.vector
                teng.tensor_mul(t[:], votes[:, g * MD:(g + 1) * MD], ncv_bc[:])
                nc.vector.tensor_reduce(
                    scores[:, g * MLOC:(g + 1) * MLOC],
                    t.rearrange("p (m d) -> p m d", d=D)[:],
                    axis=AX, op=ADD,
                )
                # keep the PE HAM clock-gate warm through the DVE-bound
                # stretch: a junk matmul paced by each group's product tile
                jp = vp.tile([128, 512], f32, tag="vp", name=f"warm{g}")
                nc.tensor.matmul(jp[:16, :], ones16[:], t[:, :512],
                                 start=True, stop=True)

            # ---- softmax over m (global over all shards via AllReduce) ----
            ev = sb.tile([128, G * MLOC], f32, tag="ev")
            nc.scalar.activation(ev[:], scores[:], EXP)
            zp = sb.tile([128, G], f32, tag="zp")
            nc.vector.tensor_reduce(
                zp[:], ev.rearrange("p (g m) -> p g m", m=MLOC)[:], axis=AX, op=ADD
            )
            zin = dram.tile([128, G], f32, tag="zin")
            zout = dram.tile([128, G], f32, tag="zout")
            nc.sync.dma_start(zin[:], zp[:])
            nc.gpsimd.collective_compute(
                "AllReduce",
                ADD,
                replica_groups=[list(range(NCORES))],
                ins=[zin.opt()],
                outs=[zout.opt()],
            )

            # ---- work hidden under the AllReduce ----
            # u = ev * act * cfac; votes *= u (in place, bcast over d). The
            # missing 1/z factor rides in the matmul lhsT (zdiag) post-AR.
            u = sb.tile([128, G * MLOC], f16, tag="u")
            nc.vector.scalar_tensor_tensor(
                u.rearrange("p (g m) -> p g m", m=MLOC)[:],
                ev.rearrange("p (g m) -> p g m", m=MLOC)[:],
                cfac[:],
                actr.unsqueeze(2).broadcast_to([128, G, MLOC]),
                op0=MUL, op1=MUL,
            )
            for g in range(G):
                vg3 = votes[:, g * MD:(g + 1) * MD].rearrange(
                    "p (m d) -> p m d", d=D
                )
                nc.vector.tensor_mul(
                    vg3,
                    vg3,
                    u[:, g * MLOC:(g + 1) * MLOC].unsqueeze(2).broadcast_to(
                        [128, MLOC, D]
                    ),
                )
            # HAM warmers paced by an ACT<->PE chain spanning the AR wait
            jk = sb.tile([128, 512], f16, tag="jk")
            nc.scalar.copy(jk[:], ev[:, :512])
            for i in range(14):
                jp = vp.tile([128, 512], f32, tag="vp", name=f"arwarm{i}")
                nc.tensor.matmul(jp[:16, :], ones16[:], jk[:, :512],
                                 start=True, stop=True)
                nc.scalar.copy(jk[:16, :], jp[:16, :])

            zt = sb.tile([128, G], f32, tag="zt")
            nc.sync.dma_start(zt[:], zout[:])
            zr = sb.tile([128, G], f32, tag="zr")
            nc.vector.reciprocal(zr[:], zt[:])

            # qk = ev * zr (bcast over m) * cfac  (cfac folds next_act/(.+eps))
            qk = sb.tile([128, G * MLOC], f32, tag="qk")
            zr_b = zr.unsqueeze(2).broadcast_to([128, G, MLOC])
            nc.vector.scalar_tensor_tensor(
                qk.rearrange("p (g m) -> p g m", m=MLOC)[:],
                ev.rearrange("p (g m) -> p g m", m=MLOC)[:],
                cfac[:],
                zr_b,
                op0=MUL, op1=MUL,
            )
            nc.sync.dma_start(qk_d[:], qk.rearrange("p (g m) -> p g m", m=MLOC)[:])

            # ---- weighted reduce: out[b,m,d] = sum_n zr * votes' where
            # votes' = votes*ev*act*cfac; zdiag[(k,b), b'] = d(b,b') zr[kb,g]
            zdiag = sb.tile([128, G * MLOC], f16, tag="zdiag")
            nc.vector.tensor_mul(
                zdiag.rearrange("p (g m) -> p g m", m=MLOC)[:],
                ones16.unsqueeze(1).broadcast_to([128, G, MLOC]),
                zr_b,
            )
            for g in range(G):
                for c in range(2):
                    nc.tensor.matmul(
                        outp[:, c * 512:(c + 1) * 512],
                        zdiag[:, g * MLOC:(g + 1) * MLOC],
                        votes[:, g * MD + c * 512: g * MD + (c + 1) * 512],
                        start=(g == 0),
                        stop=(g == G - 1),
                    )

            osb = sb.tile([16, MD], f32, tag="osb")
            nc.vector.tensor_copy(osb[:], outp[:])
            nc.sync.dma_start(out_d.rearrange("b m d -> b (m d)"), osb[:])

    nc.finalize()
    return nc


def _get_nc():
    if "nc" not in _CACHE:
        _CACHE["nc"] = _build_nc()
    return _CACHE["nc"]


def _host_inputs(x, act, W):
    """Build per-core device input maps."""
    x = np.asarray(x, np.float32)
    act = np.asarray(act, np.float32)
    W = np.asarray(W, np.float32)

    # W[n,a,m,d] -> per-core [g, a, (k, m, d)], n = 8g + k, m = 16*core + m_loc
    W6 = W.reshape(G, K8, A, NCORES, MLOC, D)
    Wt = np.ascontiguousarray(W6.transpose(3, 0, 2, 1, 4, 5), dtype=F16)
    Wt = Wt.reshape(NCORES, G, A, K8 * MD)

    # dense x with (g, k, b) on the free dim: xall[a, 128g+16k+b] = x[b, 8g+k, a]
    xall = np.ascontiguousarray(x.transpose(2, 1, 0), dtype=F16).reshape(A, G * 128)

    ones16 = np.tile(np.eye(16, dtype=F16), (K8, 1))
    rep16 = np.tile(np.eye(16, dtype=F16), (1, K8))
    actr = np.ascontiguousarray(
        act.reshape(B, G, K8).transpose(2, 0, 1), dtype=np.float32
    ).reshape(128, G)
    c = act.mean(axis=1)
    cfac = (c.astype(np.float64) / (c.astype(np.float64) + 1e-10)).astype(np.float32)
    cfac_t = np.tile(cfac, K8).reshape(128, 1)

    common = {
        "xall": xall, "ones16": ones16, "rep16": rep16,
        "actr": actr, "cfac": cfac_t,
    }
    in_maps = [{"w": np.ascontiguousarray(Wt[cid]), **common} for cid in range(NCORES)]
    return in_maps, c


def kernel(input, current_act, W, num_iter=1, **_unused):
    from concourse.bass_utils import run_bass_kernel_spmd

    in_maps, c = _host_inputs(input, current_act, W)
    nc = _get_nc()
    res = run_bass_kernel_spmd(nc, in_maps, list(range(NCORES)))

    next_capsule_value = np.concatenate(
        [res.results[cid]["out_sh"] for cid in range(NCORES)], axis=1
    ).astype(np.float32, copy=False)
    qk = np.concatenate(
        [
            res.results[cid]["qk_sh"].transpose(1, 2, 0, 3).reshape(B, NIN, MLOC)
            for cid in range(NCORES)
        ],
        axis=2,
    ).astype(np.float32, copy=False)
    next_act = np.ascontiguousarray(
        np.broadcast_to(c[:, None].astype(np.float32), (B, MTOT))
    )
    return (next_capsule_value, next_act, qk)
